# revision 17
# baseline (speedup 1.0000x reference)
"""BiChain kernel, it7: contiguous fp8 loads + col-tiled PE transposes.

Same math as it4/it5 (fp8 G matmuls with x32 weight scale, 2-step Jacobi in
[c, b] layout, halfi combine, transposed [40, B] output un-transposed on the
host).  Loads are contiguous t-major fp8 cast DMAs (4KB DRAM reads).  Each
[128 rows x 128 d] tile is transposed on the PE as FOUR concurrent 32-column
matmuls (tile_position col packing): the stationary is only 32 wide, so
LDWEIGHTS is ~27ns instead of ~97ns and the four matmuls overlap in the
array.  PSUM->SBUF copy-back alternates between DVE and ACT to halve the
per-engine copy load.
"""

import os
import sys

sys.path.insert(0, "/opt/trn_rl_repo")

import numpy as np

B, D, C = 32768, 1024, 40
C2 = 2 * C
N_CORES = 8
BS = B // N_CORES          # 4096 rows per core
P = 128
NKC = D // P               # 8 contraction chunks
NT = BS // P               # 32 row-tiles per core
WSCALE = 32.0              # fp8 weight pre-scale (undone in the sigmoid)

_CACHE = {}


def _host_prep(W, b, W_rev, b_rev):
    import ml_dtypes

    bf16 = ml_dtypes.bfloat16
    fp8 = ml_dtypes.float8_e3m4
    Wr = W_rev[::-1].copy()
    br = b_rev[::-1].copy()
    iu = np.arange(C)
    Uf = np.where(iu[None, :] < iu[:, None], W[:, D : D + C], 0.0).astype(np.float32)
    Ur = np.where(iu[None, :] > iu[:, None], Wr[:, D + C - 1 - iu], 0.0).astype(
        np.float32
    )
    Wd = np.concatenate([W[:, :D], Wr[:, :D]], axis=0)       # [80, 1024]
    wtp = np.zeros((NKC, P, P), np.float32)
    wtp[:, :, :C2] = np.ascontiguousarray(Wd.T).reshape(NKC, P, C2)
    wt = (wtp * WSCALE).transpose(1, 0, 2).reshape(P, NKC * P).astype(fp8)
    u2t = np.zeros((C2, P), np.float32)
    u2t[:C, :C] = Uf.T
    u2t[C:, C:C2] = Ur.T
    u2t = (u2t * WSCALE).astype(bf16)
    bvec = np.concatenate([b, br]).reshape(C2, 1).astype(np.float32)
    halfi = np.zeros((C2, P), np.float32)
    halfi[iu, iu] = 0.5
    halfi[C + iu, iu] = 0.5
    halfi = halfi.astype(np.float16)
    ident = np.eye(P, dtype=np.float32).astype(fp8)
    return {"wt": wt, "u2t": u2t, "bvec": bvec, "halfi": halfi, "ident": ident}


def build_nc():
    from concourse import bacc, mybir
    from concourse.tile import TileContext

    dt = mybir.dt
    AF = mybir.ActivationFunctionType

    nc = bacc.Bacc(None, target_bir_lowering=False, debug=False)
    src = nc.declare_dram_parameter("src", [BS, D], dt.float32, isOutput=False)
    wt = nc.declare_dram_parameter("wt", [P, NKC * P], dt.float8e3, isOutput=False)
    u2t = nc.declare_dram_parameter("u2t", [C2, P], dt.bfloat16, isOutput=False)
    bvec = nc.declare_dram_parameter("bvec", [C2, 1], dt.float32, isOutput=False)
    halfi = nc.declare_dram_parameter("halfi", [C2, P], dt.float16, isOutput=False)
    ident = nc.declare_dram_parameter("ident", [P, P], dt.float8e3, isOutput=False)
    out = nc.declare_dram_parameter("out", [C, BS], dt.float32, isOutput=True)

    with TileContext(nc) as tc:
        with (
            tc.tile_pool(name="const", bufs=1) as cpool,
            tc.tile_pool(name="big", bufs=1) as bigpool,
            tc.tile_pool(name="sa", bufs=2) as sapool,
            tc.tile_pool(name="sf", bufs=2) as sfpool,
            tc.tile_pool(name="ot", bufs=2) as otpool,
            tc.tile_pool(name="pst", bufs=3, space="PSUM") as pstpool,
            tc.tile_pool(name="psg", bufs=3, space="PSUM") as psgpool,
            tc.tile_pool(name="pso", bufs=2, space="PSUM") as psopool,
        ):
            wt_sb = cpool.tile([P, NKC, P], dt.float8e3)
            nc.sync.dma_start(
                out=wt_sb[:], in_=wt[:].rearrange("p (k c) -> p k c", k=NKC)
            )
            u2t_sb = cpool.tile([C2, P], dt.bfloat16)
            nc.sync.dma_start(out=u2t_sb[:], in_=u2t[:])
            b_sb = cpool.tile([C2, 1], dt.float32)
            nc.sync.dma_start(out=b_sb[:], in_=bvec[:])
            halfi_sb = cpool.tile([C2, P], dt.float16)
            nc.sync.dma_start(out=halfi_sb[:], in_=halfi[:])
            id_sb = cpool.tile([P, P], dt.float8e3)
            nc.sync.dma_start(out=id_sb[:], in_=ident[:])

            # src_sb[p, t, d] = src[t*128 + p, d]  (fp8 cast, 4KB DRAM reads)
            src_sb = bigpool.tile([P, NT, D], dt.float8e3)
            # srcT[a, kc, t, p] = src[t*128 + p, kc*128 + a]
            srcT = bigpool.tile([P, NKC, NT, P], dt.float8e3)

            src_c = src[:].rearrange("(t p) d -> p t d", p=P)

            units = [(bg * 4, 4) for bg in range(7)] + [(28 + u, 1) for u in range(4)]
            state = {}

            def stage_a(u):
                t0, nt = units[u]
                n = P * nt
                nc.gpsimd.dma_start(
                    out=src_sb[:, t0 : t0 + nt, :],
                    in_=src_c[:, t0 : t0 + nt, :],
                )
                # PE transpose: each [128, 128] tile as four concurrent
                # 32-column matmuls (cheap LDWEIGHTS, col-group packing).
                for t in range(t0, t0 + nt):
                    for kh in range(2):
                        ps_t = pstpool.tile([P, 4, P], dt.float32, name="pst")
                        for j in range(4):
                            kc = 4 * kh + j
                            nc.tensor.matmul(
                                ps_t[:, j, :],
                                lhsT=src_sb[:, t, kc * P : (kc + 1) * P],
                                rhs=id_sb[:],
                                start=True,
                                stop=True,
                            )
                        eng = nc.vector if (t + kh) % 2 == 0 else nc.scalar
                        if eng is nc.vector:
                            eng.tensor_copy(
                                srcT[:, 4 * kh : 4 * (kh + 1), t, :], ps_t[:]
                            )
                        else:
                            eng.copy(
                                srcT[:, 4 * kh : 4 * (kh + 1), t, :], ps_t[:]
                            )
                ps_g = psgpool.tile([P, 512], dt.float32, name="psg")
                for kc in range(NKC):
                    nc.tensor.matmul(
                        ps_g[:, :n],
                        lhsT=wt_sb[:, kc, :],
                        rhs=srcT[:, kc, t0 : t0 + nt, :],
                        start=(kc == 0),
                        stop=(kc == NKC - 1),
                    )
                s_a = sapool.tile([C2, 512], dt.bfloat16, name="sa")
                nc.scalar.activation(
                    out=s_a[:, :n], in_=ps_g[:C2, :n], func=AF.Sigmoid,
                    bias=b_sb[:], scale=1.0 / WSCALE,
                )
                state[u] = (ps_g, s_a)

            def stage_b(u):
                t0, nt = units[u]
                n = P * nt
                ps_g, s_a = state.pop(u)
                nc.tensor.matmul(
                    ps_g[:, :n],
                    lhsT=u2t_sb[:],
                    rhs=s_a[:, :n],
                    start=False,
                    stop=True,
                    skip_group_check=True,
                )
                sfin = sfpool.tile([C2, 512], dt.float16, name="sf")
                nc.scalar.activation(
                    out=sfin[:, :n], in_=ps_g[:C2, :n], func=AF.Sigmoid,
                    bias=b_sb[:], scale=1.0 / WSCALE,
                )
                ps_o = psopool.tile([P, 512], dt.float32, name="pso")
                nc.tensor.matmul(
                    ps_o[:, :n], lhsT=halfi_sb[:], rhs=sfin[:, :n], start=True,
                    stop=True,
                )
                ot = otpool.tile([C, 512], dt.float32, name="ot")
                nc.scalar.copy(ot[:, :n], ps_o[:C, :n])
                nc.sync.dma_start(
                    out=out[:, P * t0 : P * t0 + n], in_=ot[:, :n]
                )

            for u in range(len(units) + 1):
                if u < len(units):
                    stage_a(u)
                if u >= 1:
                    stage_b(u - 1)

    nc.compile()
    return nc


def _get_nc():
    if "nc" not in _CACHE:
        _CACHE["nc"] = build_nc()
    return _CACHE["nc"]


def _ensure_axon_hooks():
    """bass_utils imports antenv.axon_hooks when tracing; this image lacks it."""
    if "antenv.axon_hooks" in sys.modules:
        return
    import types

    mod = types.ModuleType("antenv.axon_hooks")
    mod._hook = None
    mod.set_axon_ntff_profile_hook = lambda h: setattr(mod, "_hook", h)
    mod.get_axon_ntff_profile_hook = lambda: mod._hook
    sys.modules["antenv.axon_hooks"] = mod
    try:
        from trn_agent_boot.trn_boot import _ntff_profile_via_ctypes

        mod.set_axon_ntff_profile_hook(
            _ntff_profile_via_ctypes("/opt/axon/libaxon_pjrt.so")
        )
    except Exception:
        pass


def kernel(src, attn_mask, W, b, W_rev, b_rev, **_ignored):
    _ensure_axon_hooks()
    from concourse import bass_utils

    src = np.ascontiguousarray(np.asarray(src, dtype=np.float32))
    W = np.asarray(W, dtype=np.float32)
    b = np.asarray(b, dtype=np.float32)
    W_rev = np.asarray(W_rev, dtype=np.float32)
    b_rev = np.asarray(b_rev, dtype=np.float32)

    prep = _host_prep(W, b, W_rev, b_rev)
    nc = _get_nc()

    in_maps = []
    for c in range(N_CORES):
        m = dict(prep)
        m["src"] = src[c * BS : (c + 1) * BS]
        in_maps.append(m)

    res = bass_utils.run_bass_kernel_spmd(nc, in_maps, core_ids=list(range(N_CORES)))
    outT = np.concatenate([res.results[i]["out"] for i in range(N_CORES)], axis=1)
    return np.ascontiguousarray(outT.T).astype(np.float32)
